# revision 27
# baseline (speedup 1.0000x reference)
"""Distributed causal attention (softmax over the QUERY axis) on 8 TRN2
NeuronCores, written in Bass/Tile.

Sharding: the reference normalizes softmax over the query axis (axis=1), so
each key-column's softmax is independent. Core pair (2b, 2b+1) handles batch
b, with even cores owning even 128-row k-tiles and odd cores owning odd
k-tiles; the host sums the two partial PV outputs per pair.

The two score-side weights are merged on the host (M^T = 32 Wk^T Wq) and the
merged projection is applied on the K side: kh^T = M k^T (phase K). Because
the k rows are exactly the per-core shard, each core projects only its own k
rows and the scores matmul contracts kh against RAW q (an input) - no
cross-core exchange, no collectives, no fence. Phases run back-to-back on
the PE: K (fp8 DoubleRow) -> D scores+exp (fp8 DR) -> B v-projection (bf16)
-> E PV (bf16). Inputs are host-packed partition-major; the three DMA-issue
engines (sync/scalar/gpsimd) stream inputs in consumption order and the
output tiles rotate across all three rings.
"""

from contextlib import ExitStack

import numpy as np
import ml_dtypes

import concourse.bass as bass
from concourse import bacc
import concourse.tile as tile
import concourse.mybir as mybir
from concourse.bass_utils import run_bass_kernel_spmd
from concourse.tile import ScopedClock

BATCH = 4


def _fast_drain_and_barrier(self, tick_clock, wait_clock):
    """Tile kernel-tail with sem-only all-engine barriers (the default
    drain+butterfly pair costs ~8us); the explicit sync.drain with the global
    clock waits already covers all tracked work."""
    drain_inst = self.nc.sync.drain()
    wait_clock.add_sem_waits(
        drain_inst.ins, ScopedClock({None: tick_clock.global_clock})
    )
    self.nc.all_engine_barrier(sem_only=True)
    assert self.sems is not None
    popped = self.nc._tile_sem_poison_stack.pop()
    assert popped is self._sem_poison
    # No explicit end-of-kernel semaphore clear: the lowering's own epilogue
    # already resets every engine's semaphore block, and each execution
    # starts from a fresh NEFF load, so the extra gpsimd range-clear here is
    # redundant work.
    self.nc.all_engine_barrier(sem_only=True)


tile.TileContext._drain_and_barrier = _fast_drain_and_barrier



P = 128
SEQ = 2048
E = 1024
H = 1024
KL = 1024          # k columns per core (16 tiles / 2 cores * 128)
NE = E // P        # 8
NEP = NE // 2      # 4 pair-steps for DoubleRow contraction
NH = H // P        # 8
NKS = KL // P      # 8 k slots per core
NQT = SEQ // P     # 16 q tiles
NB = 512           # matmul free-dim / psum bank
MASK_NEG = -51200.0  # pre-exp-scale; exp applies 1/1024 -> effective -50
N_WARMUP = 18

BF16 = mybir.dt.bfloat16
FP8 = mybir.dt.float8e4
F32 = mybir.dt.float32
nbf16 = ml_dtypes.bfloat16
nfp8 = ml_dtypes.float8_e4m3


def slot_chunks(j):
    """(ext, [(off, width), ...]) q-chunks for score slot j (relative to 256j)."""
    ext = SEQ - 256 * j
    chunks = []
    off = 0
    if j % 2 == 1:
        chunks.append((0, 256))
        off = 256
    while off < ext:
        chunks.append((off, NB))
        off += NB
    return ext, chunks


def build_nc():
    nc = bacc.Bacc("TRN2", target_bir_lowering=False, debug=False, num_devices=8)
    # Inputs arrive pre-packed partition-major so each tensor loads with DMAs
    # of 128 large contiguous descriptors. fp8 tensors are additionally
    # pair-interleaved over e ([P, 2, NEP*X]: partition p, free eo, ep*X+x <->
    # logical row (2*ep+eo)*128+p, col x) so the pair dim sits at AP dim 1 as
    # DoubleRow requires.
    qT2 = nc.dram_tensor("qT2", [P, 2, NEP * SEQ], FP8, kind="ExternalInput").ap()
    kT2 = nc.dram_tensor("kT2", [P, 2, NEP * KL], FP8, kind="ExternalInput").ap()
    mtT = nc.dram_tensor("mtT", [P, 2, NEP * H], FP8, kind="ExternalInput").ap()
    vT = nc.dram_tensor("vT", [P, NE * KL], BF16, kind="ExternalInput").ap()
    wvT = nc.dram_tensor("wvT", [P, NE * H], BF16, kind="ExternalInput").ap()
    mask = nc.dram_tensor("mask", [P, 256], F32, kind="ExternalInput").ap()
    out = nc.dram_tensor("out", [SEQ, H], BF16, kind="ExternalOutput").ap()

    with tile.TileContext(nc) as tc, ExitStack() as ctx:
        wpool = ctx.enter_context(tc.tile_pool(name="w", bufs=1))
        ktv = ctx.enter_context(tc.tile_pool(name="ktv", bufs=1))
        khpool = ctx.enter_context(tc.tile_pool(name="kh", bufs=NEP))
        vhpool = ctx.enter_context(tc.tile_pool(name="vh", bufs=NKS))
        prpool = ctx.enter_context(tc.tile_pool(name="pr", bufs=1))
        smpool = ctx.enter_context(tc.tile_pool(name="sm", bufs=1))
        ostpool = ctx.enter_context(tc.tile_pool(name="ost", bufs=10))
        psum = ctx.enter_context(tc.tile_pool(name="ps", bufs=8, space="PSUM"))

        # ---- PE warmup: dummy matmuls with no input deps run during the
        # initial DMA wait, releasing the HAM clock throttle early ----
        wrm = smpool.tile([P, 256], BF16, tag="wrm", name="wrm")
        nc.vector.memset(wrm[:], 0.0)
        wps = psum.tile([P, 256], F32, tag="ps", name="wps")
        for i in range(N_WARMUP):
            nc.tensor.matmul(wps[:], lhsT=wrm[:, :P], rhs=wrm[:],
                             start=(i == 0), stop=(i == N_WARMUP - 1))

        # ---- input DMAs in consumption order ----
        # Phase-K inputs stream on all four DMA-issue rings: each DR
        # accumulation round (ep) needs 4 chunks (mt/kt x eo), one per ring,
        # so the PE is never starved after the first round lands.
        msk = smpool.tile([P, 256], F32, tag="msk", name="msk")
        mt_sb = wpool.tile([P, 2, NEP * H], FP8, tag="mt", name="mt_sb")
        kt_sb = ktv.tile([P, 2, NEP * KL], FP8, tag="kt", name="kt_sb")
        qt_sb = ktv.tile([P, 2, NEP * SEQ], FP8, tag="qt", name="qt_sb")
        # Need-ordered schedule: the first DR round's four operand chunks land
        # on three rings at once (gpsimd carries ep0's kt as two 64KB kc
        # halves), so phase K starts one ring-cadence earlier; the remaining
        # chunks follow in consumption order round-robin.
        sy, sc, gp = nc.sync, nc.scalar, nc.gpsimd

        def mt_c(ring, ep, eo):
            ring.dma_start(
                mt_sb[:, eo:eo + 1, ep * H:(ep + 1) * H],
                mtT[:, eo:eo + 1, ep * H:(ep + 1) * H],
            )

        def kt_c(ring, ep, eo, a=0, b=KL):
            ring.dma_start(
                kt_sb[:, eo:eo + 1, ep * KL + a:ep * KL + b],
                kT2[:, eo:eo + 1, ep * KL + a:ep * KL + b],
            )

        mt_c(sy, 0, 0); mt_c(sc, 0, 1)
        kt_c(gp, 0, 0, 0, NB); kt_c(gp, 0, 1, 0, NB)
        kt_c(sy, 0, 0, NB, KL); kt_c(sc, 0, 1, NB, KL)
        mt_c(gp, 1, 0); mt_c(sy, 1, 1); kt_c(sc, 1, 0); kt_c(gp, 1, 1)
        mt_c(sy, 2, 0); mt_c(sc, 2, 1); kt_c(gp, 2, 0); kt_c(sy, 2, 1)
        mt_c(sc, 3, 0); mt_c(gp, 3, 1); kt_c(sy, 3, 0); kt_c(sc, 3, 1)
        gp.dma_start(msk[:], mask[:])

        # qt (phase D) then vt/wv (phase B) chunks round-robin on three rings
        wv_sb = wpool.tile([P, NE * H], BF16, tag="wv", name="wv_sb")
        vt_sb = ktv.tile([P, NE * KL], BF16, tag="vt", name="vt_sb")
        rings = [nc.sync, nc.scalar, nc.gpsimd]
        ni = 0
        for ep in range(NEP):
            for eo in range(2):
                rings[ni % 3].dma_start(
                    qt_sb[:, eo:eo + 1, ep * SEQ:(ep + 1) * SEQ],
                    qT2[:, eo:eo + 1, ep * SEQ:(ep + 1) * SEQ],
                )
                ni += 1
        chunks_b = []
        for e in range(NE):
            chunks_b.append((vt_sb, vT, e * KL, (e + 1) * KL))
            chunks_b.append((wv_sb, wvT, e * H, (e + 1) * H))
        for dst, src, a, b in chunks_b:
            rings[ni % 3].dma_start(dst[:, a:b], src[:, a:b])
            ni += 1

        # ---- phase K: kh^T[e, k_own] = M k^T in fp8 DoubleRow ----
        # e-tile-outer over 8 concurrent PSUM banks; ep is the accumulation
        # loop so matmul #1 only needs the first mt/kt chunks to have landed.
        kh_sb = [khpool.tile([P, 2, KL], FP8, tag="kh", name=f"kh{m}")
                 for m in range(NEP)]
        for kc in range(KL // NB):
            pts = [psum.tile([P, NB], F32, tag="ps", name=f"pp_kh{et}_{kc}")
                   for et in range(NE)]
            for ep in range(NEP):
                for et in range(NE):
                    nc.tensor.matmul(
                        pts[et][:],
                        lhsT=mt_sb[:, :, ep * H + et * P:ep * H + (et + 1) * P],
                        rhs=kt_sb[:, :, ep * KL + kc * NB:ep * KL + (kc + 1) * NB],
                        start=(ep == 0),
                        stop=(ep == NEP - 1),
                        perf_mode=mybir.MatmulPerfMode.DoubleRow,
                    )
            for et in range(NE):
                m, eo = et // 2, et % 2
                dst = kh_sb[m][:, eo:eo + 1, kc * NB:(kc + 1) * NB]
                if kc == 0:
                    nc.vector.tensor_copy(dst, pts[et][:])
                else:
                    # second-half casts go to ACT (idle until phase D) so the
                    # DVE queue is clear for D's mask-adds the moment the
                    # first score chunk lands
                    nc.scalar.activation(
                        dst, pts[et][:], mybir.ActivationFunctionType.Copy
                    )

        # ---- phase D: scoresT -> exp -> den (rec saved for vh scaling) ----
        pr_sb = []
        rec_sb = []
        for j in range(NKS):
            ext, chunks = slot_chunks(j)
            q0 = 256 * j
            pr = prpool.tile([P, ext], BF16, tag=f"pr{j}", name=f"pr{j}")
            accs = smpool.tile([P, len(chunks)], F32, tag=f"acc{j}", name=f"acc{j}")
            for ci, (off, w) in enumerate(chunks):
                pt = psum.tile([P, NB], F32, tag="ps", name=f"sp{j}_{ci}")
                for m in range(NEP):
                    nc.tensor.matmul(
                        pt[:, :w],
                        lhsT=kh_sb[m][:, :, j * P:(j + 1) * P],
                        rhs=qt_sb[:, :, m * SEQ + q0 + off:m * SEQ + q0 + off + w],
                        start=(m == 0),
                        stop=(m == NEP - 1),
                        perf_mode=mybir.MatmulPerfMode.DoubleRow,
                    )
                if off == 0:
                    nc.vector.tensor_add(pt[:, :256], pt[:, :256], msk[:])
                nc.scalar.activation(
                    pr[:, off:off + w],
                    pt[:, :w],
                    mybir.ActivationFunctionType.Exp,
                    scale=float(1.0 / (np.sqrt(H) * 32.0)),
                    accum_out=accs[:, ci:ci + 1],
                )
            den = smpool.tile([P, 1], F32, tag=f"den{j}", name=f"den{j}")
            nc.vector.tensor_reduce(
                den[:], accs[:], axis=mybir.AxisListType.X, op=mybir.AluOpType.add
            )
            rec = smpool.tile([P, 1], F32, tag=f"rec{j}", name=f"rec{j}")
            nc.vector.reciprocal(rec[:], den[:])
            pr_sb.append(pr)
            rec_sb.append(rec)

        # ---- phases B+E interleaved per k-slot ----
        # B(j): vh[j] via one 8-deep accumulation chain per h-bank, psum read
        # fused with the per-k-row softmax normalization (x rec). Then E
        # emits the two q-tiles (2j, 2j+1) that become complete once slot j
        # exists. Output production is thereby spread over the whole B+E
        # span, so the out-DMA rings never back up and the kernel tail is
        # just the last tile's transfer. PSUM->SBUF copies alternate
        # DVE/ACT; out-DMAs alternate two rings.
        vh_sb = [vhpool.tile([P, H], BF16, tag="vh", name=f"vh{j}")
                 for j in range(NKS)]
        n_out = 0
        for j in range(NKS):
            for hb in range(H // NB):
                pt = psum.tile([P, NB], F32, tag="ps", name=f"pp_vh{j}_{hb}")
                for e in range(NE):
                    nc.tensor.matmul(
                        pt[:],
                        lhsT=vt_sb[:, e * KL + j * P:e * KL + (j + 1) * P],
                        rhs=wv_sb[:, e * H + hb * NB:e * H + (hb + 1) * NB],
                        start=(e == 0),
                        stop=(e == NE - 1),
                    )
                nc.vector.tensor_scalar_mul(
                    vh_sb[j][:, hb * NB:(hb + 1) * NB], pt[:], rec_sb[j][:]
                )
            for t in (2 * j, 2 * j + 1):
                jmax = t // 2
                for hb in range(H // NB):
                    pt = psum.tile([P, NB], F32, tag="ps", name=f"pv{t}_{hb}")
                    for jj in range(jmax + 1):
                        off = t * P - 256 * jj
                        nc.tensor.matmul(
                            pt[:],
                            lhsT=pr_sb[jj][:, off:off + P],
                            rhs=vh_sb[jj][:, hb * NB:(hb + 1) * NB],
                            start=(jj == 0),
                            stop=(jj == jmax),
                        )
                    ot = ostpool.tile([P, NB], BF16, tag="ost", name=f"ot{t}_{hb}")
                    if n_out == 2 * NQT - 1:
                        # final tile: halve the copy and the transfer across
                        # both engines/rings so the kernel-tail flush is short
                        hw = NB // 2
                        nc.vector.tensor_copy(ot[:, :hw], pt[:, :hw])
                        nc.scalar.activation(
                            ot[:, hw:], pt[:, hw:],
                            mybir.ActivationFunctionType.Copy,
                        )
                        nc.sync.dma_start(
                            out[t * P:(t + 1) * P, hb * NB:hb * NB + hw],
                            ot[:, :hw],
                        )
                        nc.gpsimd.dma_start(
                            out[t * P:(t + 1) * P, hb * NB + hw:(hb + 1) * NB],
                            ot[:, hw:],
                        )
                    else:
                        if n_out % 2 == 0:
                            nc.vector.tensor_copy(ot[:], pt[:])
                        else:
                            nc.scalar.activation(
                                ot[:], pt[:], mybir.ActivationFunctionType.Copy
                            )
                        (nc.sync if n_out % 2 == 0 else nc.gpsimd).dma_start(
                            out[t * P:(t + 1) * P, hb * NB:(hb + 1) * NB], ot[:]
                        )
                    n_out += 1

    nc.compile()
    return nc


# ---------------- host-side prep ----------------

def core_k_tiles(parity):
    return list(range(parity, 16, 2))


def _pack(m):
    """[NE*128, X] -> [128, NE*X]: partition-major so DMA descriptors are
    large and contiguous (row e*128+p, col x) -> (p, e*X+x)."""
    r, x = m.shape
    return np.ascontiguousarray(
        m.reshape(NE, P, x).transpose(1, 0, 2).reshape(P, NE * x)
    )


def _pack_pair(m):
    """[NE*128, X] -> [128, 2, (NE/2)*X] fp8 pair-interleave for DoubleRow:
    (p, eo, ep*X+x) <-> row (2*ep+eo)*128+p, col x."""
    r, x = m.shape
    return np.ascontiguousarray(
        m.reshape(NEP, 2, P, x).transpose(2, 1, 0, 3).reshape(P, 2, NEP * x)
    )


def make_in_maps(q, k, v, Wq, Wk, Wv):
    """q,k,v: [4, 2048, 1024] f32; W*: [1024, 1024] f32 -> 8 per-core in_maps."""
    # merge the two score-side weights, applied on the k side:
    # scores = q (Wq^T Wk) k^T = q (M^T k^T)^T... with M^T = 32 Wk^T Wq
    MT = (Wk.T.astype(np.float64) @ Wq.astype(np.float64) * 32.0)
    mtT = _pack_pair(MT.astype(np.float32).astype(nfp8))
    wvT = _pack(Wv.T.astype(nbf16))

    kk = np.arange(P)[:, None]
    qq = np.arange(P)[None, :]
    tri = np.where(qq >= kk, 0.0, MASK_NEG).astype(np.float32)
    mask_even = np.concatenate([tri, np.zeros((P, P), np.float32)], axis=1)
    mask_odd = np.concatenate([np.full((P, P), MASK_NEG, np.float32), tri], axis=1)

    in_maps = []
    for c in range(8):
        b, parity = c // 2, c % 2
        rows = np.concatenate(
            [np.arange(g * P, (g + 1) * P) for g in core_k_tiles(parity)]
        )
        in_maps.append({
            "qT2": _pack_pair(q[b].T.astype(nfp8)),
            "kT2": _pack_pair(k[b][rows].T.astype(nfp8)),
            "mtT": mtT,
            "vT": _pack(v[b][rows].T.astype(nbf16)),
            "wvT": wvT,
            "mask": mask_even if parity == 0 else mask_odd,
        })
    return in_maps


def combine_outputs(outs):
    """outs: list of 8 [2048, 1024] partial arrays -> [4, 2048, 1024]."""
    res = np.empty((4, SEQ, H), np.float32)
    for b in range(4):
        res[b] = outs[2 * b].astype(np.float32) + outs[2 * b + 1].astype(np.float32)
    return res


_NC_CACHE = []


def kernel(q, k, v, Wq, Wk, Wv):
    """Full inputs in, full output out; 8-core TRN2 SPMD inside."""
    q = np.asarray(q, dtype=np.float32)
    k = np.asarray(k, dtype=np.float32)
    v = np.asarray(v, dtype=np.float32)
    Wq = np.asarray(Wq, dtype=np.float32)
    Wk = np.asarray(Wk, dtype=np.float32)
    Wv = np.asarray(Wv, dtype=np.float32)

    if not _NC_CACHE:
        _NC_CACHE.append(build_nc())
    nc = _NC_CACHE[0]

    in_maps = make_in_maps(q, k, v, Wq, Wk, Wv)
    res = run_bass_kernel_spmd(nc, in_maps, core_ids=list(range(8)))
    outs = [res.results[i]["out"] for i in range(8)]
    return combine_outputs(outs)


# revision 28
# speedup vs baseline: 1.2214x; 1.2214x over previous
"""Distributed causal attention (softmax over the QUERY axis) on 8 TRN2
NeuronCores, written in Bass/Tile.

Sharding: the reference normalizes softmax over the query axis (axis=1), so
each key-column's softmax is independent. Core pair (2b, 2b+1) handles batch
b, with even cores owning even 128-row k-tiles and odd cores owning odd
k-tiles; the host sums the two partial PV outputs per pair.

The two score-side weights are merged on the host (M^T = 32 Wk^T Wq) and the
merged projection is applied on the K side: kh^T = M k^T (phase K). Because
the k rows are exactly the per-core shard, each core projects only its own k
rows and the scores matmul contracts kh against RAW q (an input) - no
cross-core exchange, no collectives, no fence. Phases run back-to-back on
the PE: K (fp8 DoubleRow) -> D scores+exp (fp8 DR) -> B v-projection (bf16)
-> E PV (bf16). Inputs are host-packed partition-major; the three DMA-issue
engines (sync/scalar/gpsimd) stream inputs in consumption order and the
output tiles rotate across all three rings.
"""

from contextlib import ExitStack

import numpy as np
import ml_dtypes

import concourse.bass as bass
from concourse import bacc
import concourse.tile as tile
import concourse.mybir as mybir
from concourse.bass_utils import run_bass_kernel_spmd
from concourse.tile import ScopedClock

BATCH = 4


def _fast_drain_and_barrier(self, tick_clock, wait_clock):
    """Tile kernel-tail with sem-only all-engine barriers (the default
    drain+butterfly pair costs ~8us); the explicit sync.drain with the global
    clock waits already covers all tracked work."""
    drain_inst = self.nc.sync.drain()
    wait_clock.add_sem_waits(
        drain_inst.ins, ScopedClock({None: tick_clock.global_clock})
    )
    self.nc.all_engine_barrier(sem_only=True)
    assert self.sems is not None
    popped = self.nc._tile_sem_poison_stack.pop()
    assert popped is self._sem_poison
    # No explicit end-of-kernel semaphore clear: the lowering's own epilogue
    # already resets every engine's semaphore block, and each execution
    # starts from a fresh NEFF load, so the extra gpsimd range-clear here is
    # redundant work.
    self.nc.all_engine_barrier(sem_only=True)


tile.TileContext._drain_and_barrier = _fast_drain_and_barrier



P = 128
SEQ = 2048
E = 1024
H = 1024
KL = 1024          # k columns per core (16 tiles / 2 cores * 128)
NE = E // P        # 8
NEP = NE // 2      # 4 pair-steps for DoubleRow contraction
NH = H // P        # 8
NKS = KL // P      # 8 k slots per core
NQT = SEQ // P     # 16 q tiles
NB = 512           # matmul free-dim / psum bank
MASK_NEG = -51200.0  # pre-exp-scale; exp applies 1/1024 -> effective -50
N_WARMUP = 18

BF16 = mybir.dt.bfloat16
FP8 = mybir.dt.float8e4
F32 = mybir.dt.float32
nbf16 = ml_dtypes.bfloat16
nfp8 = ml_dtypes.float8_e4m3


def slot_chunks(j):
    """(ext, [(off, width), ...]) q-chunks for score slot j (relative to 256j)."""
    ext = SEQ - 256 * j
    chunks = []
    off = 0
    if j % 2 == 1:
        chunks.append((0, 256))
        off = 256
    while off < ext:
        chunks.append((off, NB))
        off += NB
    return ext, chunks


def build_nc():
    nc = bacc.Bacc("TRN2", target_bir_lowering=False, debug=False, num_devices=8)
    # Inputs arrive pre-packed partition-major so each tensor loads with DMAs
    # of 128 large contiguous descriptors. fp8 tensors are additionally
    # pair-interleaved over e ([P, 2, NEP*X]: partition p, free eo, ep*X+x <->
    # logical row (2*ep+eo)*128+p, col x) so the pair dim sits at AP dim 1 as
    # DoubleRow requires.
    qT2 = nc.dram_tensor("qT2", [P, 2, NEP * SEQ], FP8, kind="ExternalInput").ap()
    kT2 = nc.dram_tensor("kT2", [P, 2, NEP * KL], FP8, kind="ExternalInput").ap()
    mtT = nc.dram_tensor("mtT", [P, 2, NEP * H], FP8, kind="ExternalInput").ap()
    vT = nc.dram_tensor("vT", [P, NE * KL], BF16, kind="ExternalInput").ap()
    wvT = nc.dram_tensor("wvT", [P, NE * H], BF16, kind="ExternalInput").ap()
    mask = nc.dram_tensor("mask", [P, 256], F32, kind="ExternalInput").ap()
    out = nc.dram_tensor("out", [SEQ, H], BF16, kind="ExternalOutput").ap()

    with tile.TileContext(nc) as tc, ExitStack() as ctx:
        wpool = ctx.enter_context(tc.tile_pool(name="w", bufs=1))
        ktv = ctx.enter_context(tc.tile_pool(name="ktv", bufs=1))
        khpool = ctx.enter_context(tc.tile_pool(name="kh", bufs=NEP))
        vhpool = ctx.enter_context(tc.tile_pool(name="vh", bufs=NKS))
        prpool = ctx.enter_context(tc.tile_pool(name="pr", bufs=1))
        smpool = ctx.enter_context(tc.tile_pool(name="sm", bufs=1))
        ostpool = ctx.enter_context(tc.tile_pool(name="ost", bufs=10))
        psum = ctx.enter_context(tc.tile_pool(name="ps", bufs=8, space="PSUM"))

        # ---- PE warmup: dummy matmuls with no input deps run during the
        # initial DMA wait, releasing the HAM clock throttle early ----
        wrm = smpool.tile([P, 256], BF16, tag="wrm", name="wrm")
        nc.vector.memset(wrm[:], 0.0)
        wps = psum.tile([P, 256], F32, tag="ps", name="wps")
        for i in range(N_WARMUP):
            nc.tensor.matmul(wps[:], lhsT=wrm[:, :P], rhs=wrm[:],
                             start=(i == 0), stop=(i == N_WARMUP - 1))

        # ---- input DMAs in consumption order ----
        # Phase-K inputs stream on all four DMA-issue rings: each DR
        # accumulation round (ep) needs 4 chunks (mt/kt x eo), one per ring,
        # so the PE is never starved after the first round lands.
        msk = smpool.tile([P, 256], F32, tag="msk", name="msk")
        mt_sb = wpool.tile([P, 2, NEP * H], FP8, tag="mt", name="mt_sb")
        kt_sb = ktv.tile([P, 2, NEP * KL], FP8, tag="kt", name="kt_sb")
        qt_sb = ktv.tile([P, 2, NEP * SEQ], FP8, tag="qt", name="qt_sb")
        # Need-ordered schedule: the first DR round's four operand chunks land
        # on three rings at once (gpsimd carries ep0's kt as two 64KB kc
        # halves), so phase K starts one ring-cadence earlier; the remaining
        # chunks follow in consumption order round-robin.
        sy, sc, gp = nc.sync, nc.scalar, nc.gpsimd

        def mt_c(ring, ep, eo):
            ring.dma_start(
                mt_sb[:, eo:eo + 1, ep * H:(ep + 1) * H],
                mtT[:, eo:eo + 1, ep * H:(ep + 1) * H],
            )

        def kt_c(ring, ep, eo, a=0, b=KL):
            ring.dma_start(
                kt_sb[:, eo:eo + 1, ep * KL + a:ep * KL + b],
                kT2[:, eo:eo + 1, ep * KL + a:ep * KL + b],
            )

        mt_c(sy, 0, 0); mt_c(sc, 0, 1)
        kt_c(gp, 0, 0, 0, NB); kt_c(gp, 0, 1, 0, NB)
        kt_c(sy, 0, 0, NB, KL); kt_c(sc, 0, 1, NB, KL)
        mt_c(gp, 1, 0); mt_c(sy, 1, 1); kt_c(sc, 1, 0); kt_c(gp, 1, 1)
        mt_c(sy, 2, 0); mt_c(sc, 2, 1); kt_c(gp, 2, 0); kt_c(sy, 2, 1)
        mt_c(sc, 3, 0); mt_c(gp, 3, 1); kt_c(sy, 3, 0); kt_c(sc, 3, 1)
        gp.dma_start(msk[:], mask[:])

        # qt (phase D) then vt/wv (phase B) chunks round-robin on three rings
        wv_sb = wpool.tile([P, NE * H], BF16, tag="wv", name="wv_sb")
        vt_sb = ktv.tile([P, NE * KL], BF16, tag="vt", name="vt_sb")
        rings = [nc.sync, nc.scalar, nc.gpsimd]
        ni = 0
        for ep in range(NEP):
            for eo in range(2):
                rings[ni % 3].dma_start(
                    qt_sb[:, eo:eo + 1, ep * SEQ:(ep + 1) * SEQ],
                    qT2[:, eo:eo + 1, ep * SEQ:(ep + 1) * SEQ],
                )
                ni += 1
        chunks_b = []
        for e in range(NE):
            chunks_b.append((vt_sb, vT, e * KL, (e + 1) * KL))
            chunks_b.append((wv_sb, wvT, e * H, (e + 1) * H))
        for dst, src, a, b in chunks_b:
            rings[ni % 3].dma_start(dst[:, a:b], src[:, a:b])
            ni += 1

        # ---- phase K: kh^T[e, k_own] = M k^T in fp8 DoubleRow ----
        # e-tile-outer over 8 concurrent PSUM banks; ep is the accumulation
        # loop so matmul #1 only needs the first mt/kt chunks to have landed.
        kh_sb = [khpool.tile([P, 2, KL], FP8, tag="kh", name=f"kh{m}")
                 for m in range(NEP)]
        for kc in range(KL // NB):
            pts = [psum.tile([P, NB], F32, tag="ps", name=f"pp_kh{et}_{kc}")
                   for et in range(NE)]
            for ep in range(NEP):
                for et in range(NE):
                    nc.tensor.matmul(
                        pts[et][:],
                        lhsT=mt_sb[:, :, ep * H + et * P:ep * H + (et + 1) * P],
                        rhs=kt_sb[:, :, ep * KL + kc * NB:ep * KL + (kc + 1) * NB],
                        start=(ep == 0),
                        stop=(ep == NEP - 1),
                        perf_mode=mybir.MatmulPerfMode.DoubleRow,
                    )
            for et in range(NE):
                m, eo = et // 2, et % 2
                nc.vector.tensor_copy(
                    kh_sb[m][:, eo:eo + 1, kc * NB:(kc + 1) * NB], pts[et][:]
                )

        # ---- phase D: scoresT -> exp -> den (rec saved for vh scaling) ----
        pr_sb = []
        rec_sb = []
        for j in range(NKS):
            ext, chunks = slot_chunks(j)
            q0 = 256 * j
            pr = prpool.tile([P, ext], BF16, tag=f"pr{j}", name=f"pr{j}")
            accs = smpool.tile([P, len(chunks)], F32, tag=f"acc{j}", name=f"acc{j}")
            for ci, (off, w) in enumerate(chunks):
                pt = psum.tile([P, NB], F32, tag="ps", name=f"sp{j}_{ci}")
                for m in range(NEP):
                    nc.tensor.matmul(
                        pt[:, :w],
                        lhsT=kh_sb[m][:, :, j * P:(j + 1) * P],
                        rhs=qt_sb[:, :, m * SEQ + q0 + off:m * SEQ + q0 + off + w],
                        start=(m == 0),
                        stop=(m == NEP - 1),
                        perf_mode=mybir.MatmulPerfMode.DoubleRow,
                    )
                if off == 0:
                    nc.vector.tensor_add(pt[:, :256], pt[:, :256], msk[:])
                nc.scalar.activation(
                    pr[:, off:off + w],
                    pt[:, :w],
                    mybir.ActivationFunctionType.Exp,
                    scale=float(1.0 / (np.sqrt(H) * 32.0)),
                    accum_out=accs[:, ci:ci + 1],
                )
            den = smpool.tile([P, 1], F32, tag=f"den{j}", name=f"den{j}")
            nc.vector.tensor_reduce(
                den[:], accs[:], axis=mybir.AxisListType.X, op=mybir.AluOpType.add
            )
            rec = smpool.tile([P, 1], F32, tag=f"rec{j}", name=f"rec{j}")
            nc.vector.reciprocal(rec[:], den[:])
            pr_sb.append(pr)
            rec_sb.append(rec)

        # ---- phases B+E interleaved per k-slot ----
        # B(j): vh[j] via one 8-deep accumulation chain per h-bank, psum read
        # fused with the per-k-row softmax normalization (x rec). Then E
        # emits the two q-tiles (2j, 2j+1) that become complete once slot j
        # exists. Output production is thereby spread over the whole B+E
        # span, so the out-DMA rings never back up and the kernel tail is
        # just the last tile's transfer. PSUM->SBUF copies alternate
        # DVE/ACT; out-DMAs alternate two rings.
        vh_sb = [vhpool.tile([P, H], BF16, tag="vh", name=f"vh{j}")
                 for j in range(NKS)]
        n_out = 0
        for j in range(NKS):
            for hb in range(H // NB):
                pt = psum.tile([P, NB], F32, tag="ps", name=f"pp_vh{j}_{hb}")
                for e in range(NE):
                    nc.tensor.matmul(
                        pt[:],
                        lhsT=vt_sb[:, e * KL + j * P:e * KL + (j + 1) * P],
                        rhs=wv_sb[:, e * H + hb * NB:e * H + (hb + 1) * NB],
                        start=(e == 0),
                        stop=(e == NE - 1),
                    )
                nc.vector.tensor_scalar_mul(
                    vh_sb[j][:, hb * NB:(hb + 1) * NB], pt[:], rec_sb[j][:]
                )
            for t in (2 * j, 2 * j + 1):
                jmax = t // 2
                for hb in range(H // NB):
                    pt = psum.tile([P, NB], F32, tag="ps", name=f"pv{t}_{hb}")
                    for jj in range(jmax + 1):
                        off = t * P - 256 * jj
                        nc.tensor.matmul(
                            pt[:],
                            lhsT=pr_sb[jj][:, off:off + P],
                            rhs=vh_sb[jj][:, hb * NB:(hb + 1) * NB],
                            start=(jj == 0),
                            stop=(jj == jmax),
                        )
                    ot = ostpool.tile([P, NB], BF16, tag="ost", name=f"ot{t}_{hb}")
                    if n_out == 2 * NQT - 1:
                        # final tile: halve the copy and the transfer across
                        # both engines/rings so the kernel-tail flush is short
                        hw = NB // 2
                        nc.vector.tensor_copy(ot[:, :hw], pt[:, :hw])
                        nc.scalar.activation(
                            ot[:, hw:], pt[:, hw:],
                            mybir.ActivationFunctionType.Copy,
                        )
                        nc.sync.dma_start(
                            out[t * P:(t + 1) * P, hb * NB:hb * NB + hw],
                            ot[:, :hw],
                        )
                        nc.gpsimd.dma_start(
                            out[t * P:(t + 1) * P, hb * NB + hw:(hb + 1) * NB],
                            ot[:, hw:],
                        )
                    else:
                        if n_out % 2 == 0:
                            nc.vector.tensor_copy(ot[:], pt[:])
                        else:
                            nc.scalar.activation(
                                ot[:], pt[:], mybir.ActivationFunctionType.Copy
                            )
                        (nc.sync if n_out % 2 == 0 else nc.gpsimd).dma_start(
                            out[t * P:(t + 1) * P, hb * NB:(hb + 1) * NB], ot[:]
                        )
                    n_out += 1

    nc.compile()
    return nc


# ---------------- host-side prep ----------------

def core_k_tiles(parity):
    return list(range(parity, 16, 2))


def _pack(m):
    """[NE*128, X] -> [128, NE*X]: partition-major so DMA descriptors are
    large and contiguous (row e*128+p, col x) -> (p, e*X+x)."""
    r, x = m.shape
    return np.ascontiguousarray(
        m.reshape(NE, P, x).transpose(1, 0, 2).reshape(P, NE * x)
    )


def _pack_pair(m):
    """[NE*128, X] -> [128, 2, (NE/2)*X] fp8 pair-interleave for DoubleRow:
    (p, eo, ep*X+x) <-> row (2*ep+eo)*128+p, col x."""
    r, x = m.shape
    return np.ascontiguousarray(
        m.reshape(NEP, 2, P, x).transpose(2, 1, 0, 3).reshape(P, 2, NEP * x)
    )


def make_in_maps(q, k, v, Wq, Wk, Wv):
    """q,k,v: [4, 2048, 1024] f32; W*: [1024, 1024] f32 -> 8 per-core in_maps."""
    # merge the two score-side weights, applied on the k side:
    # scores = q (Wq^T Wk) k^T = q (M^T k^T)^T... with M^T = 32 Wk^T Wq
    MT = (Wk.T.astype(np.float64) @ Wq.astype(np.float64) * 32.0)
    mtT = _pack_pair(MT.astype(np.float32).astype(nfp8))
    wvT = _pack(Wv.T.astype(nbf16))

    kk = np.arange(P)[:, None]
    qq = np.arange(P)[None, :]
    tri = np.where(qq >= kk, 0.0, MASK_NEG).astype(np.float32)
    mask_even = np.concatenate([tri, np.zeros((P, P), np.float32)], axis=1)
    mask_odd = np.concatenate([np.full((P, P), MASK_NEG, np.float32), tri], axis=1)

    in_maps = []
    for c in range(8):
        b, parity = c // 2, c % 2
        rows = np.concatenate(
            [np.arange(g * P, (g + 1) * P) for g in core_k_tiles(parity)]
        )
        in_maps.append({
            "qT2": _pack_pair(q[b].T.astype(nfp8)),
            "kT2": _pack_pair(k[b][rows].T.astype(nfp8)),
            "mtT": mtT,
            "vT": _pack(v[b][rows].T.astype(nbf16)),
            "wvT": wvT,
            "mask": mask_even if parity == 0 else mask_odd,
        })
    return in_maps


def combine_outputs(outs):
    """outs: list of 8 [2048, 1024] partial arrays -> [4, 2048, 1024]."""
    res = np.empty((4, SEQ, H), np.float32)
    for b in range(4):
        res[b] = outs[2 * b].astype(np.float32) + outs[2 * b + 1].astype(np.float32)
    return res


_NC_CACHE = []


def kernel(q, k, v, Wq, Wk, Wv):
    """Full inputs in, full output out; 8-core TRN2 SPMD inside."""
    q = np.asarray(q, dtype=np.float32)
    k = np.asarray(k, dtype=np.float32)
    v = np.asarray(v, dtype=np.float32)
    Wq = np.asarray(Wq, dtype=np.float32)
    Wk = np.asarray(Wk, dtype=np.float32)
    Wv = np.asarray(Wv, dtype=np.float32)

    if not _NC_CACHE:
        _NC_CACHE.append(build_nc())
    nc = _NC_CACHE[0]

    in_maps = make_in_maps(q, k, v, Wq, Wk, Wv)
    res = run_bass_kernel_spmd(nc, in_maps, core_ids=list(range(8)))
    outs = [res.results[i]["out"] for i in range(8)]
    return combine_outputs(outs)
